# revision 27
# baseline (speedup 1.0000x reference)
"""Trainium2 Bass kernel for masked multi-head attention (B=4, S=1024, D=1024, H=16).

Sharding: 8 cores; core c handles batch b=c//2, query rows [r*512,(r+1)*512) with
r=c%2. No collectives: K/V projection work is duplicated within each core pair
(cheaper than an all-reduce on this fabric — a quad AllGather of the projected
K/V measured ~40-85us, which would sit on the critical path). All matmuls run
in bfloat16 (same PE rate as tf32, half the DMA/SBUF footprint; inputs
are converted to bf16 on the host — measured rel err 4.7e-3 vs the 2e-2
gate, with f32 PSUM accumulation and an f32 softmax denominator/output path).

Layouts (per core), everything transposed on the host so contraction dims land on
SBUF partitions:
  xtq [D, 512]  = queries[b, rows].T          xtk/xtv [D, SK] = keys/values[b,:SK].T
  wq, wo [D, D] natural
  vmask [128, NK]: vmask[p,t] = 1.0 if t*128+p < valid_len[b] else 0.0

Pipeline (all phases overlap-scheduled):
  * wq/xtq DMAs interleaved per k-tile; Q^T projection runs k-outer with 3
    concurrent PSUM accumulators so matmuls start on the first 0.75MB of DMA
    instead of waiting for the full 6MB prefix.
  * K^T projection k-outer in (m-group, column) passes: pass 0 streams behind
    the xtk DMA. xtq/xtk/xtv share a 2-deep SBUF ring so the xtv DMA is not
    serialized behind Kproj's last read.
  * V is projected into a head-interleaved store [sk, 16*(64+1)] with a vmask
    column per head: the O^T = V_aug^T @ P^T matmul yields the attention
    output rows (0..63) AND the masked softmax denominator (row 64) in one
    accumulation. Masking is purely multiplicative via the zeroed V rows
    (exp(NEG)==0 in the reference, identical result).
  * The first 16 heads' score matmuls + exp activations are interleaved into
    the K-projection passes (as each head's kt column chunk completes) and
    the V-projection tile loop (closed single-matmul groups only — holding
    an O accumulation group open across other groups miscomputes on
    hardware), spreading the ScalarE exp-throughput bound across the whole
    kernel; their P^T tiles stay resident in bf16 and the O^T passes run
    consecutively after the V loop.
  * Scores are computed transposed (S^T[sk,sq] = K_h^T-tile @ Q_h^T) with
    exp(x/8) fused on ScalarE during the PSUM->SBUF copy.
  * O-path reform (K_OREFORM=1, default): O is computed per (head, sq-tile)
    as out[sq,65] = P^T-tile^T @ V_aug (the P^T tile is the stationary
    operand, N=65 streamed columns) instead of O^T[65,sq] with N=512 and
    only 65 live output partitions. Cuts the O-phase from 49k to 29k PE
    columns (inc. transposes). The softmax division becomes a per-partition
    tensor_scalar_mul against the denominator column (no gpsimd broadcast),
    then head pairs are PE-transposed back into ot's [dout, sq] layout.
    Measured -13.8us (paired A/B) over the O^T formulation.
  * Output projection accumulates per m-tile; out DMAs are batched (4,3,1)
    m-tiles per descriptor so issue latency amortizes and the final DMA is
    a single small tile, shrinking the end-of-kernel drain. outT is written
    bf16 (K_OBF=1) to halve the drain traffic; rel err 5.0e-3 vs 4.7e-3.
  * Weight pool double-buffered (K_WB=2): the next iteration's wq DMA
    prefetches during the current iteration's attention/output phases
    instead of serializing behind outproj's last wo read at the iteration
    boundary. Measured -10us (paired A/B) on steady-state throughput.

Rejected via paired A/B (kept in-tree, K_EXCH=1): halving the duplicated
K/V projections by exchanging projected halves over a pair AllGather
(HBM bounce). PE columns drop 242k->193k but the measured result is a
wash (+4us median): the collective's flight time plus the bunched exp
(ScalarE is the bound once scores can't interleave into the projection
phases) eats the entire saving. Matches the prior session's finding that
collectives on this fabric sit on the critical path.

PE budget (nk=6, per core, 0.4167ns/column bf16): Qproj 32.8k, Kproj 49.2k,
Vproj 49.2k, scores 49.2k, O 25.0k+4.1k transposes, outproj 32.8k
= 242k columns = 100.9us tensor floor; ScalarE exp total ~41us (hidden).

Phase-ablation marginals (cumulative slope, ablate.py) vs that model:
qt +6.8u, kt +22.7u (~model), v +29.1u (+8.6: DVE mask-multiply lag),
attn +55.2u (+22.6: ScalarE exp back-pressure through the 2-deep score
PSUM ring stalls the in-order PE queue), out +49.9u (+36: outproj gates
on the full ot store, absorbing the pair loop's DVE/ScalarE tail).
PSUM banks rebalanced to psA=3/psS=3/psO=2 (-3.9us median): a deeper
score ring softens the exp back-pressure; the O-reform's psO tiles are
tiny but each pool buf still rounds to a full 2KB bank. Moving the
softmax division (K_SDIV) or outproj drains (K_SOUT) to ScalarE measured
neutral-to-worse - the per-op PSUM access penalty (~172 cycles) negates
the engine rebalance; both ship disabled.

Score drip (K_DRIP=1) + ot split (K_OTSPLIT=1), -4.8us median: instead of
dumping each Kproj pass's ready scores in one burst (24 exps = 10.2us of
ScalarE vs 5.1us of pass PE -> in-order PE queue stalls on the score
PSUM ring), ready (head, tile) pairs are queued and dripped at most 12
per Kproj pass boundary and 5 per Vproj (t, half) step, with a flush
before the pair loop. All emission stays at closed-accumulation-group
boundaries (interleaving singles into an open group miscomputes on HW).
The attention store is split into 4 [128, 2*SQ] tiles so outproj's entry
dependency is per pair-of-pairs rather than the full store.
"""

import os
import ml_dtypes
import numpy as np

BF16_NP = ml_dtypes.bfloat16

import concourse.bass as bass
import concourse.tile as tile
from concourse import bacc, masks, mybir
from concourse.bass_utils import run_bass_kernel_spmd

B, S, D = 4, 1024, 1024
H, HD = 16, 64
N_CORES = 8
SQ = 512  # query rows per core
F32 = mybir.dt.float32
F32R = mybir.dt.float32r
BF16 = mybir.dt.bfloat16
VW = 65  # per-head v_store width (64 dims + 1 mask/ones column)

_module_cache: dict[int, object] = {}


def _build_module_exch(nk: int, reps: int = 1):
    """Pair-exchange variant: each core projects only HALF its batch's key
    tiles for K and V; the core pair exchanges the projected halves via an
    HBM AllGather, halving the duplicated K/V projection work. Tile order in
    kt/vs is global (rank-major), so all scores wait on the gathered tiles;
    the Q projection plus its interleaved score/exp emission overlaps the
    collective's flight time. Odd nk is padded to even with a zero-masked
    tile (host supplies zero keys/mask for it)."""
    nkp = nk + (nk % 2)
    hk = nkp // 2
    sk2 = hk * 128
    skp = nkp * 128
    nkt = D // 128
    nm = D // 128
    W = nm * sk2 + hk * H * VW  # exchange payload columns (bf16)

    nc = bacc.Bacc("TRN2", target_bir_lowering=False, debug=False,
                   num_devices=N_CORES)

    xtq_d = nc.dram_tensor("xtq", [D, SQ], BF16, kind="ExternalInput")
    xtk_d = nc.dram_tensor("xtk", [D, sk2], BF16, kind="ExternalInput")
    xtv_d = nc.dram_tensor("xtv", [D, sk2], BF16, kind="ExternalInput")
    wq_d = nc.dram_tensor("wq", [D, D], BF16, kind="ExternalInput")
    wo_d = nc.dram_tensor("wo", [D, D], BF16, kind="ExternalInput")
    vm_d = nc.dram_tensor("vmask", [128, hk], F32, kind="ExternalInput")
    out_d = nc.dram_tensor("outT", [D, SQ], F32, kind="ExternalOutput")

    with tile.TileContext(nc) as tc:
        with (
            tc.tile_pool(name="w", bufs=1) as wpool,
            tc.tile_pool(name="xtkv", bufs=3) as xtkvpool,
            tc.tile_pool(name="stg", bufs=2) as stgpool,
            tc.tile_pool(name="qt", bufs=1) as qtpool,
            tc.tile_pool(name="kt", bufs=1) as ktpool,
            tc.tile_pool(name="vs", bufs=1) as vspool,
            tc.tile_pool(name="pt", bufs=int(os.environ.get("K_PTB", "16"))) as ptpool,
            tc.tile_pool(name="ot", bufs=1) as otpool,
            tc.tile_pool(name="small", bufs=1) as smallpool,
            tc.tile_pool(name="opa", bufs=int(os.environ.get("K_OPA", "3"))) as opapool,
            tc.tile_pool(name="inv", bufs=int(os.environ.get("K_INV", "2"))) as invpool,
            tc.tile_pool(name="osb", bufs=int(os.environ.get("K_OSB", "2"))) as osbpool,
            tc.tile_pool(name="dram", bufs=2, space="DRAM") as drampool,
            tc.tile_pool(name="psA", bufs=int(os.environ.get("K_PSA", "3")), space="PSUM") as psA,
            tc.tile_pool(name="psS", bufs=int(os.environ.get("K_PSS", "3")), space="PSUM") as psS,
            tc.tile_pool(name="psO", bufs=int(os.environ.get("K_PSO", "2")), space="PSUM") as psO,
        ):
          def emit_rep():
              # weights + xtk first so the Kproj half starts the exchange asap
              wq_sb = wpool.tile([128, nkt * D], BF16, tag="w")
              xtk_sb = xtkvpool.tile([128, nkt * sk2], BF16, tag="xtkv",
                                     name="xtk")
              for k in range(nkt):
                  nc.sync.dma_start(out=wq_sb[:, k * D:(k + 1) * D],
                                    in_=wq_d.ap()[k * 128:(k + 1) * 128, :])
                  nc.sync.dma_start(out=xtk_sb[:, k * sk2:(k + 1) * sk2],
                                    in_=xtk_d.ap()[k * 128:(k + 1) * 128, :])
              vmask_sb = smallpool.tile([128, hk], F32, tag="vmask")
              nc.sync.dma_start(out=vmask_sb[:], in_=vm_d.ap())
              ones16 = smallpool.tile([128, 16], F32, tag="ones16")
              nc.vector.memset(ones16[:], 1.0)
              ident = smallpool.tile([128, 128], BF16, tag="ident")
              masks.make_identity(nc, ident[:])
              xtv_sb = xtkvpool.tile([128, nkt * sk2], BF16, tag="xtkv",
                                     name="xtv")
              for k in range(nkt):
                  nc.sync.dma_start(out=xtv_sb[:, k * sk2:(k + 1) * sk2],
                                    in_=xtv_d.ap()[k * 128:(k + 1) * 128, :])
              xtq_sb = xtkvpool.tile([128, nkt * SQ], BF16, tag="xtkv",
                                     name="xtq")
              for k in range(nkt):
                  nc.sync.dma_start(out=xtq_sb[:, k * SQ:(k + 1) * SQ],
                                    in_=xtq_d.ap()[k * 128:(k + 1) * 128, :])

              # ---- K^T half projection into staging
              ktx_sb = stgpool.tile([128, nm * sk2], BF16, tag="stg",
                                    name="ktx")
              gw = int(os.environ.get("K_PSA", "3"))
              mgroups = [list(range(o, min(o + gw, nm))) for o in range(0, nm, gw)]
              for ms in mgroups:
                  pss = {m: psA.tile([128, sk2], F32, tag="proj", name=f"ka{m}")
                         for m in ms}
                  for k in range(nkt):
                      for m in ms:
                          nc.tensor.matmul(
                              pss[m][:],
                              wq_sb[:, k * D + m * 128: k * D + (m + 1) * 128],
                              xtk_sb[:, k * sk2:(k + 1) * sk2],
                              start=(k == 0), stop=(k == nkt - 1))
                  for m in ms:
                      nc.vector.tensor_copy(ktx_sb[:, m * sk2:(m + 1) * sk2],
                                            pss[m][:])

              # ---- V half projection into staging (head-interleaved + mask)
              vsx_sb = stgpool.tile([128, hk * H * VW], BF16, tag="stg",
                                    name="vsx")
              for t in range(hk):
                  for half in range(2):
                      ps = psA.tile([128, 512], F32, tag="proj")
                      for k in range(nkt):
                          nc.tensor.matmul(
                              ps[:],
                              xtv_sb[:, k * sk2 + t * 128: k * sk2 + (t + 1) * 128],
                              wq_sb[:, k * D + half * 512: k * D + half * 512 + 512],
                              start=(k == 0), stop=(k == nkt - 1))
                      dst = vsx_sb[:, t * H * VW + half * 8 * VW:
                                   t * H * VW + (half + 1) * 8 * VW]
                      dst = dst.rearrange("p (h c) -> p h c", c=VW)[:, :, 0:HD]
                      src = ps[:].rearrange("p (h c) -> p h c", c=HD)
                      nc.vector.tensor_scalar_mul(dst, src, vmask_sb[:, t:t + 1])
                      mcols = vsx_sb[:, t * H * VW: (t + 1) * H * VW]
                      mcols = mcols.rearrange("p (h c) -> p h c", c=VW)
                      mcols = mcols[:, half * 8:(half + 1) * 8, HD:VW]
                      o16 = ones16[:].rearrange("p (h o) -> p h o", o=1)
                      nc.vector.tensor_scalar_mul(
                          mcols, o16[:, half * 8:(half + 1) * 8, :],
                          vmask_sb[:, t:t + 1])

              # ---- exchange: stage to DRAM, pair AllGather, pull both halves
              kvx_mine = drampool.tile([128, W], BF16, tag="kvm")
              kvx_both = drampool.tile([2 * 128, W], BF16, tag="kvb")
              nc.sync.dma_start(out=kvx_mine[:, 0:nm * sk2], in_=ktx_sb[:])
              nc.sync.dma_start(out=kvx_mine[:, nm * sk2:W], in_=vsx_sb[:])
              nc.gpsimd.collective_compute(
                  "AllGather", mybir.AluOpType.bypass,
                  replica_groups=[[0, 1], [2, 3], [4, 5], [6, 7]],
                  ins=[kvx_mine.opt()], outs=[kvx_both.opt()])
              kt_sb = ktpool.tile([128, nm * skp], BF16, tag="kt")
              vs_sb = vspool.tile([128, nkp * H * VW], BF16, tag="vs")
              for g in range(2):
                  src = kvx_both[g * 128:(g + 1) * 128, 0:nm * sk2]
                  src = src.rearrange("p (m c) -> p m c", c=sk2)
                  dst = kt_sb[:].rearrange("p (m c) -> p m c", c=skp)
                  dst = dst[:, :, g * sk2:(g + 1) * sk2]
                  nc.sync.dma_start(out=dst, in_=src)
                  nc.sync.dma_start(
                      out=vs_sb[:, g * hk * H * VW:(g + 1) * hk * H * VW],
                      in_=kvx_both[g * 128:(g + 1) * 128, nm * sk2:W])

              # ---- Q^T projection; each m-group's heads' score/exp follow it
              # so Activation starts while the collective is in flight
              qt_sb = qtpool.tile([128, nm * SQ], BF16, tag="qt")
              pt_tiles = {}

              def emit_score(h, t):
                  po = 64 * (h % 2)
                  mb = h // 2
                  ss = psS.tile([128, SQ], F32, tag="s", name=f"ss{h}_{t}")
                  nc.tensor.matmul(
                      ss[:],
                      kt_sb[po:po + 64, mb * skp + t * 128: mb * skp + (t + 1) * 128],
                      qt_sb[po:po + 64, mb * SQ:(mb + 1) * SQ],
                      start=True, stop=True)
                  nc.scalar.activation(
                      pt_tiles[h][:, t * SQ:(t + 1) * SQ], ss[:],
                      mybir.ActivationFunctionType.Exp, scale=0.125)

              for ms in mgroups:
                  pss = {m: psA.tile([128, SQ], F32, tag="proj", name=f"qa{m}")
                         for m in ms}
                  for k in range(nkt):
                      for m in ms:
                          nc.tensor.matmul(
                              pss[m][:],
                              wq_sb[:, k * D + m * 128: k * D + (m + 1) * 128],
                              xtq_sb[:, k * SQ:(k + 1) * SQ],
                              start=(k == 0), stop=(k == nkt - 1))
                  for m in ms:
                      nc.vector.tensor_copy(qt_sb[:, m * SQ:(m + 1) * SQ],
                                            pss[m][:])
                  for m in ms:
                      for h in (2 * m, 2 * m + 1):
                          pt_tiles[h] = ptpool.tile([128, nkp * SQ], BF16,
                                                    tag="pt", name=f"pt{h}")
                          for t in range(nkp):
                              emit_score(h, t)

              # wo into the wq slot after Qproj (last wq reader)
              wo_sb = wpool.tile([128, nkt * D], BF16, tag="w")
              for k in range(nkt):
                  nc.sync.dma_start(out=wo_sb[:, k * D:(k + 1) * D],
                                    in_=wo_d.ap()[k * 128:(k + 1) * 128, :])

              # ---- O + divide + transpose per head pair (as in oreform)
              ot_sb = otpool.tile([128, nm * SQ], BF16, tag="ot")
              for mb in range(nm):
                  for i in range(SQ // 128):
                      opair = opapool.tile([128, 128], BF16, tag="opair")
                      for h in (2 * mb, 2 * mb + 1):
                          ph = psO.tile([128, VW], F32, tag="o")
                          for t in range(nkp):
                              nc.tensor.matmul(
                                  ph[:],
                                  pt_tiles[h][:, t * SQ + i * 128:
                                              t * SQ + (i + 1) * 128],
                                  vs_sb[:, (t * H + h) * VW:
                                        (t * H + h + 1) * VW],
                                  start=(t == 0), stop=(t == nkp - 1))
                          inv = invpool.tile([128, 1], F32, tag="inv")
                          nc.vector.reciprocal(inv[:], ph[:, 64:65])
                          nc.vector.tensor_scalar_mul(
                              opair[:, 64 * (h % 2): 64 * (h % 2) + 64],
                              ph[:, 0:64], inv[:])
                      pst = psO.tile([128, 128], BF16, tag="o")
                      nc.tensor.transpose(pst[:], opair[:], ident[:])
                      nc.vector.tensor_copy(
                          ot_sb[:, mb * SQ + i * 128: mb * SQ + (i + 1) * 128],
                          pst[:])

              # ---- output projection; out DMAs batched (4,3,1)
              osb4 = None
              for m in range(nm):
                  ps = psA.tile([128, SQ], F32, tag="proj")
                  for k in range(nkt):
                      nc.tensor.matmul(
                          ps[:],
                          wo_sb[:, k * D + m * 128: k * D + (m + 1) * 128],
                          ot_sb[:, k * SQ:(k + 1) * SQ],
                          start=(k == 0), stop=(k == nkt - 1))
                  if m in (0, 4, 7):
                      bw = {0: 4, 4: 3, 7: 1}[m]
                      mbase = m
                      osb4 = osbpool.tile([128, 4 * SQ], F32, tag="outsb")
                  nc.vector.tensor_copy(
                      osb4[:, (m - mbase) * SQ:(m - mbase + 1) * SQ], ps[:])
                  if m == mbase + bw - 1:
                      dst = out_d.ap()[mbase * 128:(m + 1) * 128, :]
                      dst = dst.rearrange("(mm p) c -> p mm c", p=128)
                      nc.sync.dma_start(out=dst, in_=osb4[:, :bw * SQ])

          for _rep in range(reps):
              emit_rep()

    nc.compile()
    return nc


def _build_module(nk: int, reps: int = 1, phases=None, variant: int = 0):
    """Build the SPMD Bass module; nk = number of 128-row key tiles.

    reps > 1 emits the whole pipeline multiple times (same pools) — used only
    for slope-based device-time measurement in the dev harness.
    """
    def on(name):
        return phases is None or name in phases

    chunk_pt = (nk >= 7) or (variant == 1)
    # O-path reform: compute O per (head, sq-tile) as out[sq,65] with the
    # P^T tile as the stationary operand (N=65 streamed columns instead of
    # N=512 with only 65 live output partitions), divide by the denominator
    # column on Pool, then PE-transpose head pairs back into ot's
    # [dout, sq] layout. ~20k fewer PE columns than the O^T formulation.
    oreform = (os.environ.get("K_OREFORM", "1") == "1") and not chunk_pt

    sk = nk * 128
    nkt = D // 128  # contraction k-tiles for the projections
    nm = D // 128   # output row-tiles (128 douts each)

    nc = bacc.Bacc("TRN2", target_bir_lowering=False, debug=False,
                   num_devices=N_CORES)

    xtq_d = nc.dram_tensor("xtq", [D, SQ], BF16, kind="ExternalInput")
    xtk_d = nc.dram_tensor("xtk", [D, sk], BF16, kind="ExternalInput")
    xtv_d = nc.dram_tensor("xtv", [D, sk], BF16, kind="ExternalInput")
    wq_d = nc.dram_tensor("wq", [D, D], BF16, kind="ExternalInput")
    wo_d = nc.dram_tensor("wo", [D, D], BF16, kind="ExternalInput")
    vm_d = nc.dram_tensor("vmask", [128, nk], F32, kind="ExternalInput")
    obf = os.environ.get("K_OBF", "1") == "1"
    out_d = nc.dram_tensor("outT", [D, SQ], BF16 if obf else F32,
                           kind="ExternalOutput")

    with tile.TileContext(nc) as tc:
        with (
            tc.tile_pool(name="w", bufs=int(os.environ.get("K_WB", "2"))) as wpool,
            tc.tile_pool(name="xtq", bufs=1) as _xtqpool,
            tc.tile_pool(name="xtkv", bufs=int(os.environ.get("K_XTKV", "2"))) as xtkvpool,
            tc.tile_pool(name="qt", bufs=1) as qtpool,
            tc.tile_pool(name="kt", bufs=1) as ktpool,
            tc.tile_pool(name="vs", bufs=1) as vspool,
            tc.tile_pool(name="pt", bufs=(4 if ((nk >= 7) or (variant == 1)) else max(2, int(os.environ.get("K_EARLY", "16"))))) as ptpool,
            tc.tile_pool(name="ot", bufs=1) as otpool,
            tc.tile_pool(name="small", bufs=1) as smallpool,
            tc.tile_pool(name="opa", bufs=int(os.environ.get("K_OPA", "3"))) as opapool,
            tc.tile_pool(name="inv", bufs=int(os.environ.get("K_INV", "2"))) as invpool,
            tc.tile_pool(name="osb", bufs=int(os.environ.get("K_OSB", "2"))) as osbpool,
            tc.tile_pool(name="psA", bufs=int(os.environ.get("K_PSA", "3")), space="PSUM") as psA,
            tc.tile_pool(name="psS", bufs=int(os.environ.get("K_PSS", "3")), space="PSUM") as psS,
            tc.tile_pool(name="psO", bufs=int(os.environ.get("K_PSO", "2")), space="PSUM") as psO,
        ):
          for _rep in range(reps):
              # ---- resident weights (wq slot later reused for wo via same tag)
              # wq / xtq DMAs interleaved per k-tile so the k-outer Q
              # projection can start after the first pair lands.
              wq_sb = wpool.tile([128, nkt * D], BF16, tag="w")
              xtq_sb = xtkvpool.tile([128, nkt * SQ], BF16, tag="xtkv")
              for k in range(nkt):
                  nc.sync.dma_start(out=wq_sb[:, k * D:(k + 1) * D],
                                    in_=wq_d.ap()[k * 128:(k + 1) * 128, :])
                  nc.sync.dma_start(out=xtq_sb[:, k * SQ:(k + 1) * SQ],
                                    in_=xtq_d.ap()[k * 128:(k + 1) * 128, :])

              vmask_sb = smallpool.tile([128, nk], F32, tag="vmask")
              nc.sync.dma_start(out=vmask_sb[:], in_=vm_d.ap())
              ones16 = smallpool.tile([128, 16], F32, tag="ones16")
              nc.vector.memset(ones16[:], 1.0)
              if oreform:
                  ident = smallpool.tile([128, 128], BF16, tag="ident")
                  masks.make_identity(nc, ident[:])

              xtk_sb = xtkvpool.tile([128, nkt * sk], BF16, tag="xtkv")
              for k in range(nkt):
                  nc.sync.dma_start(out=xtk_sb[:, k * sk:(k + 1) * sk],
                                    in_=xtk_d.ap()[k * 128:(k + 1) * 128, :])
              xtv_sb = xtkvpool.tile([128, nkt * sk], BF16, tag="xtkv")
              for k in range(nkt):
                  nc.sync.dma_start(out=xtv_sb[:, k * sk:(k + 1) * sk],
                                    in_=xtv_d.ap()[k * 128:(k + 1) * 128, :])

              # ---- Q^T projection: qt[dout, sq]; k-outer with 4 PSUM
              # accumulators so matmuls start on the first wq/xtq k-tile.
              qt_sb = qtpool.tile([128, nm * SQ], BF16, tag="qt")
              gw = int(os.environ.get("K_PSA", "3")) if os.environ.get("K_KOUTER", "1") == "1" else 1
              mgroups = [list(range(o, min(o + gw, nm))) for o in range(0, nm, gw)]
              for ms in (mgroups if on("qt") else []):
                  pss = {m: psA.tile([128, SQ], F32, tag="proj", name=f"qa{m}") for m in ms}
                  for k in range(nkt):
                      for m in ms:
                          nc.tensor.matmul(
                              pss[m][:],
                              wq_sb[:, k * D + m * 128: k * D + (m + 1) * 128],
                              xtq_sb[:, k * SQ:(k + 1) * SQ],
                              start=(k == 0), stop=(k == nkt - 1))
                  for m in ms:
                      nc.vector.tensor_copy(qt_sb[:, m * SQ:(m + 1) * SQ], pss[m][:])

              # ---- K^T projection: kt[dout, sk]; k-outer in (m-group, col)
              # passes so pass 0 streams behind the xtk DMA. Early heads'
              # score+exp for completed kt column chunks are emitted between
              # passes so the Activation engine starts during Kproj.
              n_early = int(os.environ.get("K_EARLY", "16")) if not chunk_pt else 0
              heads_early = tuple(range(n_early)) if (on("v") and on("attn")) else ()
              pt_early = {h: ptpool.tile([128, nk * SQ], BF16, tag="pt",
                                         name=f"pte{h}") for h in heads_early}
              scores_done = set()
              # drip: bound the score/exp burst emitted at each closed-pass
              # boundary so ScalarE digests exps during the next pass's PE
              # work instead of back-pressuring the in-order PE queue.
              score_q = []
              drip_on = os.environ.get("K_DRIP", "1") == "1"
              dripk = int(os.environ.get("K_DRIPK", "12"))
              dripv = int(os.environ.get("K_DRIPV", "5"))

              def drip(n):
                  for _ in range(min(n, len(score_q))):
                      emit_score(*score_q.pop(0))

              def emit_score(h, t):
                  po = 64 * (h % 2)
                  mb = h // 2
                  ss = psS.tile([128, SQ], F32, tag="s", name=f"ss{h}_{t}")
                  nc.tensor.matmul(
                      ss[:],
                      kt_sb[po:po + 64, mb * sk + t * 128: mb * sk + (t + 1) * 128],
                      qt_sb[po:po + 64, mb * SQ:(mb + 1) * SQ],
                      start=True, stop=True)
                  nc.scalar.activation(
                      pt_early[h][:, t * SQ:(t + 1) * SQ], ss[:],
                      mybir.ActivationFunctionType.Exp, scale=0.125)
                  scores_done.add((h, t))

              kt_sb = ktpool.tile([128, nm * sk], BF16, tag="kt")
              nsplits = [(o, min(512, sk - o)) for o in range(0, sk, 512)]
              for (noff, nw) in (nsplits if on("kt") else []):
                  for ms in mgroups:
                      pss = {m: psA.tile([128, 512], F32, tag="proj", name=f"ka{m}") for m in ms}
                      for k in range(nkt):
                          for m in ms:
                              nc.tensor.matmul(
                                  pss[m][:, :nw],
                                  wq_sb[:, k * D + m * 128: k * D + (m + 1) * 128],
                                  xtk_sb[:, k * sk + noff: k * sk + noff + nw],
                                  start=(k == 0), stop=(k == nkt - 1))
                      for m in ms:
                          nc.vector.tensor_copy(
                              kt_sb[:, m * sk + noff: m * sk + noff + nw],
                              pss[m][:, :nw])
                      # score any (early head, tile) whose kt chunk is now done
                      for h in (hh for hh in heads_early if hh // 2 in ms):
                          for t in range(noff // 128,
                                         min(nk, (noff + nw) // 128)):
                              if drip_on:
                                  score_q.append((h, t))
                              else:
                                  emit_score(h, t)
                      if drip_on:
                          drip(dripk)

              # wo loads into the wq slot; Tile serializes on wq's last reader
              wo_sb = wpool.tile([128, nkt * D], BF16, tag="w")
              for k in range(nkt):
                  nc.sync.dma_start(out=wo_sb[:, k * D:(k + 1) * D],
                                    in_=wo_d.ap()[k * 128:(k + 1) * 128, :])

              # ---- V projection into head-interleaved store with mask
              # columns; heads 0/1's score/exp/O work is interleaved into the
              # tile loop so the Activation engine starts ~25us earlier.
              otsplit = oreform and os.environ.get("K_OTSPLIT", "1") == "1"
              if otsplit:
                  # 4-way split so outproj's dependency granularity on ot is
                  # per pair-of-pairs, not the whole store
                  ot_tiles = [otpool.tile([128, 2 * SQ], BF16, tag=f"ot{j}",
                                          name=f"ot{j}")
                              for j in range(nm // 2)]
              else:
                  ot_sb = otpool.tile([128, nm * SQ], BF16, tag="ot")

              def ot_ap(mb, lo, hi):
                  if otsplit:
                      return ot_tiles[mb // 2][:, (mb % 2) * SQ + lo:
                                               (mb % 2) * SQ + hi]
                  return ot_sb[:, mb * SQ + lo: mb * SQ + hi]

              vs_sb = vspool.tile([128, nk * H * VW], BF16, tag="vs")
              for t in range(nk if on("v") else 0):
                  for half in range(2):  # d columns [half*512, half*512+512)
                      ps = psA.tile([128, 512], F32, tag="proj")
                      for k in range(nkt):
                          nc.tensor.matmul(
                              ps[:],
                              xtv_sb[:, k * sk + t * 128: k * sk + (t + 1) * 128],
                              wq_sb[:, k * D + half * 512: k * D + half * 512 + 512],
                              start=(k == 0), stop=(k == nkt - 1))
                      dst = vs_sb[:, t * H * VW + half * 8 * VW:
                                  t * H * VW + (half + 1) * 8 * VW]
                      dst = dst.rearrange("p (h c) -> p h c", c=VW)[:, :, 0:HD]
                      src = ps[:].rearrange("p (h c) -> p h c", c=HD)
                      nc.vector.tensor_scalar_mul(dst, src, vmask_sb[:, t:t + 1])
                      mcols = vs_sb[:, t * H * VW: (t + 1) * H * VW]
                      mcols = mcols.rearrange("p (h c) -> p h c", c=VW)
                      mcols = mcols[:, half * 8:(half + 1) * 8, HD:VW]
                      o16 = ones16[:].rearrange("p (h o) -> p h o", o=1)
                      nc.vector.tensor_scalar_mul(
                          mcols, o16[:, half * 8:(half + 1) * 8, :],
                          vmask_sb[:, t:t + 1])
                      if drip_on:
                          drip(dripv)
                      elif half == 0:
                          for h in heads_early:
                              if (h, t) not in scores_done:
                                  emit_score(h, t)

              if drip_on:
                  drip(len(score_q))

              if oreform:
                  # ---- O + divide + transpose per head pair (2mb, 2mb+1)
                  for mb in range(nm if on("attn") else 0):
                      for h in (2 * mb, 2 * mb + 1):
                          if h not in pt_early:
                              pt_early[h] = ptpool.tile(
                                  [128, nk * SQ], BF16, tag="pt", name=f"ptl{h}")
                              for t in range(nk):
                                  emit_score(h, t)
                      for i in range(SQ // 128):
                          opair = opapool.tile([128, 128], BF16, tag="opair")
                          for h in (2 * mb, 2 * mb + 1):
                              ph = psO.tile([128, VW], F32, tag="o")
                              for t in range(nk):
                                  nc.tensor.matmul(
                                      ph[:],
                                      pt_early[h][:, t * SQ + i * 128:
                                                  t * SQ + (i + 1) * 128],
                                      vs_sb[:, (t * H + h) * VW:
                                            (t * H + h + 1) * VW],
                                      start=(t == 0), stop=(t == nk - 1))
                              inv = invpool.tile([128, 1], F32, tag="inv")
                              nc.vector.reciprocal(inv[:], ph[:, 64:65])
                              if os.environ.get("K_SDIV", "0") == "1":
                                  nc.scalar.mul(
                                      opair[:, 64 * (h % 2): 64 * (h % 2) + 64],
                                      ph[:, 0:64], inv[:])
                              else:
                                  nc.vector.tensor_scalar_mul(
                                      opair[:, 64 * (h % 2): 64 * (h % 2) + 64],
                                      ph[:, 0:64], inv[:])
                          pst = psO.tile([128, 128], BF16, tag="o")
                          nc.tensor.transpose(pst[:], opair[:], ident[:])
                          nc.vector.tensor_copy(
                              ot_ap(mb, i * 128, (i + 1) * 128), pst[:])

              for h in heads_early if not oreform else ():
                  po = 64 * (h % 2)
                  mb = h // 2
                  po_ps = psO.tile([VW, SQ], F32, tag="o")
                  for t in range(nk):
                      nc.tensor.matmul(
                          po_ps[:],
                          vs_sb[:, t * H * VW + h * VW: t * H * VW + (h + 1) * VW],
                          pt_early[h][:, t * SQ:(t + 1) * SQ],
                          start=(t == 0), stop=(t == nk - 1))
                  inv = invpool.tile([1, SQ], F32, tag="inv")
                  nc.vector.reciprocal(inv[:], po_ps[64:65, :])
                  inv_rep = invpool.tile([64, SQ], F32, tag="invrep")
                  nc.gpsimd.partition_broadcast(inv_rep[:], inv[:])
                  nc.vector.tensor_mul(
                      ot_sb[po:po + 64, mb * SQ:(mb + 1) * SQ],
                      po_ps[0:64, :], inv_rep[:])

              # ---- attention for the remaining heads
              for h in range(len(heads_early) if (on("attn") and not oreform) else H,
                             H if (on("attn") and not oreform) else 0):
                  po = 64 * (h % 2)       # partition offset of this head's douts
                  mb = h // 2             # dout row-tile holding this head
                  if not chunk_pt:
                      # P^T per head resident; score/exp pass then O^T pass
                      pt = ptpool.tile([128, nk * SQ], BF16, tag="pt")
                      for t in range(nk):
                          ss = psS.tile([128, SQ], F32, tag="s")
                          nc.tensor.matmul(
                              ss[:],
                              kt_sb[po:po + 64, mb * sk + t * 128: mb * sk + (t + 1) * 128],
                              qt_sb[po:po + 64, mb * SQ:(mb + 1) * SQ],
                              start=True, stop=True)
                          nc.scalar.activation(pt[:, t * SQ:(t + 1) * SQ], ss[:],
                                               mybir.ActivationFunctionType.Exp,
                                               scale=0.125)
                      po_ps = psO.tile([VW, SQ], F32, tag="o")
                      for t in range(nk):
                          nc.tensor.matmul(
                              po_ps[:],
                              vs_sb[:, t * H * VW + h * VW: t * H * VW + (h + 1) * VW],
                              pt[:, t * SQ:(t + 1) * SQ],
                              start=(t == 0), stop=(t == nk - 1))
                  else:
                      # chunked P^T (smaller SBUF footprint for large nk)
                      po_ps = psO.tile([VW, SQ], F32, tag="o")
                      for t in range(nk):
                          ss = psS.tile([128, SQ], F32, tag="s")
                          nc.tensor.matmul(
                              ss[:],
                              kt_sb[po:po + 64, mb * sk + t * 128: mb * sk + (t + 1) * 128],
                              qt_sb[po:po + 64, mb * SQ:(mb + 1) * SQ],
                              start=True, stop=True)
                          ptc = ptpool.tile([128, SQ], BF16, tag="pt")
                          nc.scalar.activation(ptc[:], ss[:],
                                               mybir.ActivationFunctionType.Exp,
                                               scale=0.125)
                          nc.tensor.matmul(
                              po_ps[:],
                              vs_sb[:, t * H * VW + h * VW: t * H * VW + (h + 1) * VW],
                              ptc[:],
                              start=(t == 0), stop=(t == nk - 1),
                              skip_group_check=True)
                  inv = invpool.tile([1, SQ], F32, tag="inv")
                  nc.vector.reciprocal(inv[:], po_ps[64:65, :])
                  # broadcast inv to 64 partitions on the (idle) gpsimd engine
                  inv_rep = invpool.tile([64, SQ], F32, tag="invrep")
                  nc.gpsimd.partition_broadcast(inv_rep[:], inv[:])
                  nc.vector.tensor_mul(
                      ot_sb[po:po + 64, mb * SQ:(mb + 1) * SQ],
                      po_ps[0:64, :], inv_rep[:])

              # ---- output projection: outT[dout, sq] = Wo^T-tiles @ O^T
              # out DMAs batched 4 m-tiles at a time to amortize issue latency
              osb4 = None
              for m in range(nm if on("out") else 0):
                  ps = psA.tile([128, SQ], F32, tag="proj")
                  for k in range(nkt):
                      nc.tensor.matmul(
                          ps[:],
                          wo_sb[:, k * D + m * 128: k * D + (m + 1) * 128],
                          ot_ap(k, 0, SQ),
                          start=(k == 0), stop=(k == nkt - 1))
                  if m in (0, 4, 7):
                      bw = {0: 4, 4: 3, 7: 1}[m]
                      mbase = m
                      osb4 = osbpool.tile([128, 4 * SQ],
                                          BF16 if obf else F32, tag="outsb")
                  if os.environ.get("K_SOUT", "0") == "1":
                      nc.scalar.copy(
                          osb4[:, (m - mbase) * SQ:(m - mbase + 1) * SQ], ps[:])
                  else:
                      nc.vector.tensor_copy(
                          osb4[:, (m - mbase) * SQ:(m - mbase + 1) * SQ], ps[:])
                  if m == mbase + bw - 1:
                      dst = out_d.ap()[mbase * 128:(m + 1) * 128, :]
                      dst = dst.rearrange("(mm p) c -> p mm c", p=128)
                      nc.sync.dma_start(out=dst, in_=osb4[:, :bw * SQ])

    nc.compile()
    return nc


def exch_enabled() -> bool:
    return os.environ.get("K_EXCH", "0") == "1"


def make_in_maps(queries, keys, values, vls, W_q, W_o, nk):
    """Per-core input tensors matching the active module variant."""
    sk = nk * 128
    wq16 = np.ascontiguousarray(np.asarray(W_q, np.float32).astype(BF16_NP))
    wo16 = np.ascontiguousarray(np.asarray(W_o, np.float32).astype(BF16_NP))
    exch = exch_enabled()
    nkp = nk + (nk % 2)
    hk = nkp // 2
    sk2 = hk * 128
    in_maps = []
    for c in range(N_CORES):
        b, r = c // 2, c % 2
        vl = int(vls[b])
        if exch:
            lo = r * sk2
            hi = min(sk, lo + sk2)
            xk = np.zeros((D, sk2), dtype=BF16_NP)
            xv = np.zeros((D, sk2), dtype=BF16_NP)
            if hi > lo:
                xk[:, 0:hi - lo] = keys[b, lo:hi, :].T.astype(BF16_NP)
                xv[:, 0:hi - lo] = values[b, lo:hi, :].T.astype(BF16_NP)
            vm = (np.arange(lo, lo + sk2) < vl).astype(np.float32)
            vm = vm.reshape(hk, 128).T
        else:
            xk = keys[b, :sk, :].T.astype(BF16_NP)
            xv = values[b, :sk, :].T.astype(BF16_NP)
            vm = (np.arange(sk) < vl).astype(np.float32).reshape(nk, 128).T
        in_maps.append({
            "xtq": np.ascontiguousarray(
                queries[b, r * SQ:(r + 1) * SQ, :].T.astype(BF16_NP)),
            "xtk": np.ascontiguousarray(xk),
            "xtv": np.ascontiguousarray(xv),
            "wq": wq16,
            "wo": wo16,
            "vmask": np.ascontiguousarray(vm),
        })
    return in_maps


def get_module(nk: int):
    key = (nk, exch_enabled())
    nc = _module_cache.get(key)
    if nc is None:
        nc = _build_module_exch(nk) if exch_enabled() else _build_module(nk)
        _module_cache[key] = nc
    return nc


def build_for_bench(nk: int, reps: int = 1):
    """Fresh (uncached) module for the active variant; used by test harness."""
    if exch_enabled():
        return _build_module_exch(nk, reps=reps)
    return _build_module(nk, reps=reps)


def kernel(queries, keys, values, valid_lengths, W_q, W_o):
    queries = np.ascontiguousarray(np.asarray(queries, dtype=np.float32))
    keys = np.ascontiguousarray(np.asarray(keys, dtype=np.float32))
    values = np.ascontiguousarray(np.asarray(values, dtype=np.float32))
    W_q = np.ascontiguousarray(np.asarray(W_q, dtype=np.float32))
    W_o = np.ascontiguousarray(np.asarray(W_o, dtype=np.float32))
    vls = np.asarray(valid_lengths).astype(np.int64)

    nk = max(1, int(-(-int(vls.max()) // 128)))  # ceil(max_vl/128)

    nc = get_module(nk)
    in_maps = make_in_maps(queries, keys, values, vls, W_q, W_o, nk)

    res = run_bass_kernel_spmd(nc, in_maps, list(range(N_CORES)))

    out = np.empty((B, S, D), dtype=np.float32)
    for c in range(N_CORES):
        b, r = c // 2, c % 2
        out[b, r * SQ:(r + 1) * SQ, :] = \
            res.results[c]["outT"].T.astype(np.float32)
    return out



# revision 28
# speedup vs baseline: 1.1390x; 1.1390x over previous
"""Trainium2 Bass kernel for masked multi-head attention (B=4, S=1024, D=1024, H=16).

Sharding: 8 cores; core c handles batch b=c//2, query rows [r*512,(r+1)*512) with
r=c%2. No collectives: K/V projection work is duplicated within each core pair
(cheaper than an all-reduce on this fabric — a quad AllGather of the projected
K/V measured ~40-85us, which would sit on the critical path). All matmuls run
in bfloat16 (same PE rate as tf32, half the DMA/SBUF footprint; inputs
are converted to bf16 on the host — measured rel err 4.7e-3 vs the 2e-2
gate, with f32 PSUM accumulation and an f32 softmax denominator/output path).

Layouts (per core), everything transposed on the host so contraction dims land on
SBUF partitions:
  xtq [D, 512]  = queries[b, rows].T          xtk/xtv [D, SK] = keys/values[b,:SK].T
  wq, wo [D, D] natural
  vmask [128, NK]: vmask[p,t] = 1.0 if t*128+p < valid_len[b] else 0.0

Pipeline (all phases overlap-scheduled):
  * wq/xtq DMAs interleaved per k-tile; Q^T projection runs k-outer with 3
    concurrent PSUM accumulators so matmuls start on the first 0.75MB of DMA
    instead of waiting for the full 6MB prefix.
  * K^T projection k-outer in (m-group, column) passes: pass 0 streams behind
    the xtk DMA. xtq/xtk/xtv share a 2-deep SBUF ring so the xtv DMA is not
    serialized behind Kproj's last read.
  * V is projected into a head-interleaved store [sk, 16*(64+1)] with a vmask
    column per head: the O^T = V_aug^T @ P^T matmul yields the attention
    output rows (0..63) AND the masked softmax denominator (row 64) in one
    accumulation. Masking is purely multiplicative via the zeroed V rows
    (exp(NEG)==0 in the reference, identical result).
  * The first 16 heads' score matmuls + exp activations are interleaved into
    the K-projection passes (as each head's kt column chunk completes) and
    the V-projection tile loop (closed single-matmul groups only — holding
    an O accumulation group open across other groups miscomputes on
    hardware), spreading the ScalarE exp-throughput bound across the whole
    kernel; their P^T tiles stay resident in bf16 and the O^T passes run
    consecutively after the V loop.
  * Scores are computed transposed (S^T[sk,sq] = K_h^T-tile @ Q_h^T) with
    exp(x/8) fused on ScalarE during the PSUM->SBUF copy.
  * O-path reform (K_OREFORM=1, default): O is computed per (head, sq-tile)
    as out[sq,65] = P^T-tile^T @ V_aug (the P^T tile is the stationary
    operand, N=65 streamed columns) instead of O^T[65,sq] with N=512 and
    only 65 live output partitions. Cuts the O-phase from 49k to 29k PE
    columns (inc. transposes). The softmax division becomes a per-partition
    tensor_scalar_mul against the denominator column (no gpsimd broadcast),
    then head pairs are PE-transposed back into ot's [dout, sq] layout.
    Measured -13.8us (paired A/B) over the O^T formulation.
  * Output projection accumulates per m-tile; out DMAs are batched (4,3,1)
    m-tiles per descriptor so issue latency amortizes and the final DMA is
    a single small tile, shrinking the end-of-kernel drain. outT is written
    bf16 (K_OBF=1) to halve the drain traffic; rel err 5.0e-3 vs 4.7e-3.
  * Weight pool double-buffered (K_WB=2): the next iteration's wq DMA
    prefetches during the current iteration's attention/output phases
    instead of serializing behind outproj's last wo read at the iteration
    boundary. Measured -10us (paired A/B) on steady-state throughput.

Rejected via paired A/B (kept in-tree, K_EXCH=1): halving the duplicated
K/V projections by exchanging projected halves over a pair AllGather
(HBM bounce). PE columns drop 242k->193k but the measured result is a
wash (+4us median): the collective's flight time plus the bunched exp
(ScalarE is the bound once scores can't interleave into the projection
phases) eats the entire saving. Matches the prior session's finding that
collectives on this fabric sit on the critical path.

PE budget (nk=6, per core, 0.4167ns/column bf16): Qproj 32.8k, Kproj 49.2k,
Vproj 49.2k, scores 49.2k, O 25.0k+4.1k transposes, outproj 32.8k
= 242k columns = 100.9us tensor floor; ScalarE exp total ~41us (hidden).

Phase-ablation marginals (cumulative slope, ablate.py) vs that model:
qt +6.8u, kt +22.7u (~model), v +29.1u (+8.6: DVE mask-multiply lag),
attn +55.2u (+22.6: ScalarE exp back-pressure through the 2-deep score
PSUM ring stalls the in-order PE queue), out +49.9u (+36: outproj gates
on the full ot store, absorbing the pair loop's DVE/ScalarE tail).
PSUM banks rebalanced to psA=3/psS=3/psO=2 (-3.9us median): a deeper
score ring softens the exp back-pressure; the O-reform's psO tiles are
tiny but each pool buf still rounds to a full 2KB bank. Moving the
softmax division (K_SDIV) or outproj drains (K_SOUT) to ScalarE measured
neutral-to-worse - the per-op PSUM access penalty (~172 cycles) negates
the engine rebalance; both ship disabled.

Score drip (K_DRIP=1) + ot split (K_OTSPLIT=1), -4.8us median: instead of
dumping each Kproj pass's ready scores in one burst (24 exps = 10.2us of
ScalarE vs 5.1us of pass PE -> in-order PE queue stalls on the score
PSUM ring), ready (head, tile) pairs are queued and dripped at most 12
per Kproj pass boundary and 5 per Vproj (t, half) step, with a flush
before the pair loop. All emission stays at closed-accumulation-group
boundaries (interleaving singles into an open group miscomputes on HW).
The attention store is split into 4 [128, 2*SQ] tiles so outproj's entry
dependency is per pair-of-pairs rather than the full store.
"""

import os
import ml_dtypes
import numpy as np

BF16_NP = ml_dtypes.bfloat16

import concourse.bass as bass
import concourse.tile as tile
from concourse import bacc, masks, mybir
from concourse.bass_utils import run_bass_kernel_spmd

B, S, D = 4, 1024, 1024
H, HD = 16, 64
N_CORES = 8
SQ = 512  # query rows per core
F32 = mybir.dt.float32
F32R = mybir.dt.float32r
BF16 = mybir.dt.bfloat16
VW = 65  # per-head v_store width (64 dims + 1 mask/ones column)

_module_cache: dict[int, object] = {}


def _build_module_exch(nk: int, reps: int = 1):
    """Pair-exchange variant: each core projects only HALF its batch's key
    tiles for K and V; the core pair exchanges the projected halves via an
    HBM AllGather, halving the duplicated K/V projection work. Tile order in
    kt/vs is global (rank-major), so all scores wait on the gathered tiles;
    the Q projection plus its interleaved score/exp emission overlaps the
    collective's flight time. Odd nk is padded to even with a zero-masked
    tile (host supplies zero keys/mask for it)."""
    nkp = nk + (nk % 2)
    hk = nkp // 2
    sk2 = hk * 128
    skp = nkp * 128
    nkt = D // 128
    nm = D // 128
    W = nm * sk2 + hk * H * VW  # exchange payload columns (bf16)

    nc = bacc.Bacc("TRN2", target_bir_lowering=False, debug=False,
                   num_devices=N_CORES)

    xtq_d = nc.dram_tensor("xtq", [D, SQ], BF16, kind="ExternalInput")
    xtk_d = nc.dram_tensor("xtk", [D, sk2], BF16, kind="ExternalInput")
    xtv_d = nc.dram_tensor("xtv", [D, sk2], BF16, kind="ExternalInput")
    wq_d = nc.dram_tensor("wq", [D, D], BF16, kind="ExternalInput")
    wo_d = nc.dram_tensor("wo", [D, D], BF16, kind="ExternalInput")
    vm_d = nc.dram_tensor("vmask", [128, hk], F32, kind="ExternalInput")
    out_d = nc.dram_tensor("outT", [D, SQ], F32, kind="ExternalOutput")

    with tile.TileContext(nc) as tc:
        with (
            tc.tile_pool(name="w", bufs=1) as wpool,
            tc.tile_pool(name="xtkv", bufs=3) as xtkvpool,
            tc.tile_pool(name="stg", bufs=2) as stgpool,
            tc.tile_pool(name="qt", bufs=1) as qtpool,
            tc.tile_pool(name="kt", bufs=1) as ktpool,
            tc.tile_pool(name="vs", bufs=1) as vspool,
            tc.tile_pool(name="pt", bufs=int(os.environ.get("K_PTB", "16"))) as ptpool,
            tc.tile_pool(name="ot", bufs=1) as otpool,
            tc.tile_pool(name="small", bufs=1) as smallpool,
            tc.tile_pool(name="opa", bufs=int(os.environ.get("K_OPA", "3"))) as opapool,
            tc.tile_pool(name="inv", bufs=int(os.environ.get("K_INV", "2"))) as invpool,
            tc.tile_pool(name="osb", bufs=int(os.environ.get("K_OSB", "2"))) as osbpool,
            tc.tile_pool(name="dram", bufs=2, space="DRAM") as drampool,
            tc.tile_pool(name="psA", bufs=int(os.environ.get("K_PSA", "3")), space="PSUM") as psA,
            tc.tile_pool(name="psS", bufs=int(os.environ.get("K_PSS", "3")), space="PSUM") as psS,
            tc.tile_pool(name="psO", bufs=int(os.environ.get("K_PSO", "2")), space="PSUM") as psO,
        ):
          def emit_rep():
              # weights + xtk first so the Kproj half starts the exchange asap
              wq_sb = wpool.tile([128, nkt * D], BF16, tag="w")
              xtk_sb = xtkvpool.tile([128, nkt * sk2], BF16, tag="xtkv",
                                     name="xtk")
              for k in range(nkt):
                  nc.sync.dma_start(out=wq_sb[:, k * D:(k + 1) * D],
                                    in_=wq_d.ap()[k * 128:(k + 1) * 128, :])
                  nc.sync.dma_start(out=xtk_sb[:, k * sk2:(k + 1) * sk2],
                                    in_=xtk_d.ap()[k * 128:(k + 1) * 128, :])
              vmask_sb = smallpool.tile([128, hk], F32, tag="vmask")
              nc.sync.dma_start(out=vmask_sb[:], in_=vm_d.ap())
              ones16 = smallpool.tile([128, 16], F32, tag="ones16")
              nc.vector.memset(ones16[:], 1.0)
              ident = smallpool.tile([128, 128], BF16, tag="ident")
              masks.make_identity(nc, ident[:])
              xtv_sb = xtkvpool.tile([128, nkt * sk2], BF16, tag="xtkv",
                                     name="xtv")
              for k in range(nkt):
                  nc.sync.dma_start(out=xtv_sb[:, k * sk2:(k + 1) * sk2],
                                    in_=xtv_d.ap()[k * 128:(k + 1) * 128, :])
              xtq_sb = xtkvpool.tile([128, nkt * SQ], BF16, tag="xtkv",
                                     name="xtq")
              for k in range(nkt):
                  nc.sync.dma_start(out=xtq_sb[:, k * SQ:(k + 1) * SQ],
                                    in_=xtq_d.ap()[k * 128:(k + 1) * 128, :])

              # ---- K^T half projection into staging
              ktx_sb = stgpool.tile([128, nm * sk2], BF16, tag="stg",
                                    name="ktx")
              gw = int(os.environ.get("K_PSA", "3"))
              mgroups = [list(range(o, min(o + gw, nm))) for o in range(0, nm, gw)]
              for ms in mgroups:
                  pss = {m: psA.tile([128, sk2], F32, tag="proj", name=f"ka{m}")
                         for m in ms}
                  for k in range(nkt):
                      for m in ms:
                          nc.tensor.matmul(
                              pss[m][:],
                              wq_sb[:, k * D + m * 128: k * D + (m + 1) * 128],
                              xtk_sb[:, k * sk2:(k + 1) * sk2],
                              start=(k == 0), stop=(k == nkt - 1))
                  for m in ms:
                      nc.vector.tensor_copy(ktx_sb[:, m * sk2:(m + 1) * sk2],
                                            pss[m][:])

              # ---- V half projection into staging (head-interleaved + mask)
              vsx_sb = stgpool.tile([128, hk * H * VW], BF16, tag="stg",
                                    name="vsx")
              for t in range(hk):
                  for half in range(2):
                      ps = psA.tile([128, 512], F32, tag="proj")
                      for k in range(nkt):
                          nc.tensor.matmul(
                              ps[:],
                              xtv_sb[:, k * sk2 + t * 128: k * sk2 + (t + 1) * 128],
                              wq_sb[:, k * D + half * 512: k * D + half * 512 + 512],
                              start=(k == 0), stop=(k == nkt - 1))
                      dst = vsx_sb[:, t * H * VW + half * 8 * VW:
                                   t * H * VW + (half + 1) * 8 * VW]
                      dst = dst.rearrange("p (h c) -> p h c", c=VW)[:, :, 0:HD]
                      src = ps[:].rearrange("p (h c) -> p h c", c=HD)
                      nc.vector.tensor_scalar_mul(dst, src, vmask_sb[:, t:t + 1])
                      mcols = vsx_sb[:, t * H * VW: (t + 1) * H * VW]
                      mcols = mcols.rearrange("p (h c) -> p h c", c=VW)
                      mcols = mcols[:, half * 8:(half + 1) * 8, HD:VW]
                      o16 = ones16[:].rearrange("p (h o) -> p h o", o=1)
                      nc.vector.tensor_scalar_mul(
                          mcols, o16[:, half * 8:(half + 1) * 8, :],
                          vmask_sb[:, t:t + 1])

              # ---- exchange: stage to DRAM, pair AllGather, pull both halves
              kvx_mine = drampool.tile([128, W], BF16, tag="kvm")
              kvx_both = drampool.tile([2 * 128, W], BF16, tag="kvb")
              nc.sync.dma_start(out=kvx_mine[:, 0:nm * sk2], in_=ktx_sb[:])
              nc.sync.dma_start(out=kvx_mine[:, nm * sk2:W], in_=vsx_sb[:])
              nc.gpsimd.collective_compute(
                  "AllGather", mybir.AluOpType.bypass,
                  replica_groups=[[0, 1], [2, 3], [4, 5], [6, 7]],
                  ins=[kvx_mine.opt()], outs=[kvx_both.opt()])
              kt_sb = ktpool.tile([128, nm * skp], BF16, tag="kt")
              vs_sb = vspool.tile([128, nkp * H * VW], BF16, tag="vs")
              for g in range(2):
                  src = kvx_both[g * 128:(g + 1) * 128, 0:nm * sk2]
                  src = src.rearrange("p (m c) -> p m c", c=sk2)
                  dst = kt_sb[:].rearrange("p (m c) -> p m c", c=skp)
                  dst = dst[:, :, g * sk2:(g + 1) * sk2]
                  nc.sync.dma_start(out=dst, in_=src)
                  nc.sync.dma_start(
                      out=vs_sb[:, g * hk * H * VW:(g + 1) * hk * H * VW],
                      in_=kvx_both[g * 128:(g + 1) * 128, nm * sk2:W])

              # ---- Q^T projection; each m-group's heads' score/exp follow it
              # so Activation starts while the collective is in flight
              qt_sb = qtpool.tile([128, nm * SQ], BF16, tag="qt")
              pt_tiles = {}

              def emit_score(h, t):
                  po = 64 * (h % 2)
                  mb = h // 2
                  ss = psS.tile([128, SQ], F32, tag="s", name=f"ss{h}_{t}")
                  nc.tensor.matmul(
                      ss[:],
                      kt_sb[po:po + 64, mb * skp + t * 128: mb * skp + (t + 1) * 128],
                      qt_sb[po:po + 64, mb * SQ:(mb + 1) * SQ],
                      start=True, stop=True)
                  nc.scalar.activation(
                      pt_tiles[h][:, t * SQ:(t + 1) * SQ], ss[:],
                      mybir.ActivationFunctionType.Exp, scale=0.125)

              for ms in mgroups:
                  pss = {m: psA.tile([128, SQ], F32, tag="proj", name=f"qa{m}")
                         for m in ms}
                  for k in range(nkt):
                      for m in ms:
                          nc.tensor.matmul(
                              pss[m][:],
                              wq_sb[:, k * D + m * 128: k * D + (m + 1) * 128],
                              xtq_sb[:, k * SQ:(k + 1) * SQ],
                              start=(k == 0), stop=(k == nkt - 1))
                  for m in ms:
                      nc.vector.tensor_copy(qt_sb[:, m * SQ:(m + 1) * SQ],
                                            pss[m][:])
                  for m in ms:
                      for h in (2 * m, 2 * m + 1):
                          pt_tiles[h] = ptpool.tile([128, nkp * SQ], BF16,
                                                    tag="pt", name=f"pt{h}")
                          for t in range(nkp):
                              emit_score(h, t)

              # wo into the wq slot after Qproj (last wq reader)
              wo_sb = wpool.tile([128, nkt * D], BF16, tag="w")
              for k in range(nkt):
                  nc.sync.dma_start(out=wo_sb[:, k * D:(k + 1) * D],
                                    in_=wo_d.ap()[k * 128:(k + 1) * 128, :])

              # ---- O + divide + transpose per head pair (as in oreform)
              ot_sb = otpool.tile([128, nm * SQ], BF16, tag="ot")
              for mb in range(nm):
                  for i in range(SQ // 128):
                      opair = opapool.tile([128, 128], BF16, tag="opair")
                      for h in (2 * mb, 2 * mb + 1):
                          ph = psO.tile([128, VW], F32, tag="o")
                          for t in range(nkp):
                              nc.tensor.matmul(
                                  ph[:],
                                  pt_tiles[h][:, t * SQ + i * 128:
                                              t * SQ + (i + 1) * 128],
                                  vs_sb[:, (t * H + h) * VW:
                                        (t * H + h + 1) * VW],
                                  start=(t == 0), stop=(t == nkp - 1))
                          inv = invpool.tile([128, 1], F32, tag="inv")
                          nc.vector.reciprocal(inv[:], ph[:, 64:65])
                          nc.vector.tensor_scalar_mul(
                              opair[:, 64 * (h % 2): 64 * (h % 2) + 64],
                              ph[:, 0:64], inv[:])
                      pst = psO.tile([128, 128], BF16, tag="o")
                      nc.tensor.transpose(pst[:], opair[:], ident[:])
                      nc.vector.tensor_copy(
                          ot_sb[:, mb * SQ + i * 128: mb * SQ + (i + 1) * 128],
                          pst[:])

              # ---- output projection; out DMAs batched (4,3,1)
              osb4 = None
              for m in range(nm):
                  ps = psA.tile([128, SQ], F32, tag="proj")
                  for k in range(nkt):
                      nc.tensor.matmul(
                          ps[:],
                          wo_sb[:, k * D + m * 128: k * D + (m + 1) * 128],
                          ot_sb[:, k * SQ:(k + 1) * SQ],
                          start=(k == 0), stop=(k == nkt - 1))
                  if m in (0, 4, 7):
                      bw = {0: 4, 4: 3, 7: 1}[m]
                      mbase = m
                      osb4 = osbpool.tile([128, 4 * SQ], F32, tag="outsb")
                  nc.vector.tensor_copy(
                      osb4[:, (m - mbase) * SQ:(m - mbase + 1) * SQ], ps[:])
                  if m == mbase + bw - 1:
                      dst = out_d.ap()[mbase * 128:(m + 1) * 128, :]
                      dst = dst.rearrange("(mm p) c -> p mm c", p=128)
                      nc.sync.dma_start(out=dst, in_=osb4[:, :bw * SQ])

          for _rep in range(reps):
              emit_rep()

    nc.compile()
    return nc


def _build_module(nk: int, reps: int = 1, phases=None, variant: int = 0):
    """Build the SPMD Bass module; nk = number of 128-row key tiles.

    reps > 1 emits the whole pipeline multiple times (same pools) — used only
    for slope-based device-time measurement in the dev harness.
    """
    def on(name):
        return phases is None or name in phases

    chunk_pt = (nk >= 7) or (variant == 1)
    # O-path reform: compute O per (head, sq-tile) as out[sq,65] with the
    # P^T tile as the stationary operand (N=65 streamed columns instead of
    # N=512 with only 65 live output partitions), divide by the denominator
    # column on Pool, then PE-transpose head pairs back into ot's
    # [dout, sq] layout. ~20k fewer PE columns than the O^T formulation.
    oreform = (os.environ.get("K_OREFORM", "1") == "1") and not chunk_pt

    sk = nk * 128
    nkt = D // 128  # contraction k-tiles for the projections
    nm = D // 128   # output row-tiles (128 douts each)

    nc = bacc.Bacc("TRN2", target_bir_lowering=False, debug=False,
                   num_devices=N_CORES)

    xtq_d = nc.dram_tensor("xtq", [D, SQ], BF16, kind="ExternalInput")
    xtk_d = nc.dram_tensor("xtk", [D, sk], BF16, kind="ExternalInput")
    xtv_d = nc.dram_tensor("xtv", [D, sk], BF16, kind="ExternalInput")
    wq_d = nc.dram_tensor("wq", [D, D], BF16, kind="ExternalInput")
    wo_d = nc.dram_tensor("wo", [D, D], BF16, kind="ExternalInput")
    vm_d = nc.dram_tensor("vmask", [128, nk], F32, kind="ExternalInput")
    obf = os.environ.get("K_OBF", "1") == "1"
    out_d = nc.dram_tensor("outT", [D, SQ], BF16 if obf else F32,
                           kind="ExternalOutput")

    with tile.TileContext(nc) as tc:
        with (
            tc.tile_pool(name="w", bufs=int(os.environ.get("K_WB", "2"))) as wpool,
            tc.tile_pool(name="xtq", bufs=1) as _xtqpool,
            tc.tile_pool(name="xtkv", bufs=int(os.environ.get("K_XTKV", "2"))) as xtkvpool,
            tc.tile_pool(name="qt", bufs=1) as qtpool,
            tc.tile_pool(name="kt", bufs=1) as ktpool,
            tc.tile_pool(name="vs", bufs=1) as vspool,
            tc.tile_pool(name="pt", bufs=(4 if ((nk >= 7) or (variant == 1)) else max(2, int(os.environ.get("K_EARLY", "16"))))) as ptpool,
            tc.tile_pool(name="ot", bufs=1) as otpool,
            tc.tile_pool(name="small", bufs=1) as smallpool,
            tc.tile_pool(name="opa", bufs=int(os.environ.get("K_OPA", "3"))) as opapool,
            tc.tile_pool(name="inv", bufs=int(os.environ.get("K_INV", "2"))) as invpool,
            tc.tile_pool(name="osb", bufs=int(os.environ.get("K_OSB", "2"))) as osbpool,
            tc.tile_pool(name="psA", bufs=int(os.environ.get("K_PSA", "3")), space="PSUM") as psA,
            tc.tile_pool(name="psS", bufs=int(os.environ.get("K_PSS", "3")), space="PSUM") as psS,
            tc.tile_pool(name="psO", bufs=int(os.environ.get("K_PSO", "2")), space="PSUM") as psO,
        ):
          for _rep in range(reps):
              # ---- resident weights (wq slot later reused for wo via same tag)
              # wq / xtq DMAs interleaved per k-tile so the k-outer Q
              # projection can start after the first pair lands.
              wq_sb = wpool.tile([128, nkt * D], BF16, tag="w")
              xtq_sb = xtkvpool.tile([128, nkt * SQ], BF16, tag="xtkv")
              for k in range(nkt):
                  nc.sync.dma_start(out=wq_sb[:, k * D:(k + 1) * D],
                                    in_=wq_d.ap()[k * 128:(k + 1) * 128, :])
                  nc.sync.dma_start(out=xtq_sb[:, k * SQ:(k + 1) * SQ],
                                    in_=xtq_d.ap()[k * 128:(k + 1) * 128, :])

              vmask_sb = smallpool.tile([128, nk], F32, tag="vmask")
              nc.sync.dma_start(out=vmask_sb[:], in_=vm_d.ap())
              ones16 = smallpool.tile([128, 16], F32, tag="ones16")
              nc.vector.memset(ones16[:], 1.0)
              if oreform:
                  ident = smallpool.tile([128, 128], BF16, tag="ident")
                  masks.make_identity(nc, ident[:])

              xtk_sb = xtkvpool.tile([128, nkt * sk], BF16, tag="xtkv")
              for k in range(nkt):
                  nc.sync.dma_start(out=xtk_sb[:, k * sk:(k + 1) * sk],
                                    in_=xtk_d.ap()[k * 128:(k + 1) * 128, :])
              xtv_sb = xtkvpool.tile([128, nkt * sk], BF16, tag="xtkv")
              for k in range(nkt):
                  nc.sync.dma_start(out=xtv_sb[:, k * sk:(k + 1) * sk],
                                    in_=xtv_d.ap()[k * 128:(k + 1) * 128, :])

              # ---- Q^T projection: qt[dout, sq]; k-outer with 4 PSUM
              # accumulators so matmuls start on the first wq/xtq k-tile.
              qt_sb = qtpool.tile([128, nm * SQ], BF16, tag="qt")
              gw = int(os.environ.get("K_PSA", "3")) if os.environ.get("K_KOUTER", "1") == "1" else 1
              mgroups = [list(range(o, min(o + gw, nm))) for o in range(0, nm, gw)]
              for ms in (mgroups if on("qt") else []):
                  pss = {m: psA.tile([128, SQ], F32, tag="proj", name=f"qa{m}") for m in ms}
                  for k in range(nkt):
                      for m in ms:
                          nc.tensor.matmul(
                              pss[m][:],
                              wq_sb[:, k * D + m * 128: k * D + (m + 1) * 128],
                              xtq_sb[:, k * SQ:(k + 1) * SQ],
                              start=(k == 0), stop=(k == nkt - 1))
                  for m in ms:
                      nc.vector.tensor_copy(qt_sb[:, m * SQ:(m + 1) * SQ], pss[m][:])

              # ---- K^T projection: kt[dout, sk]; k-outer in (m-group, col)
              # passes so pass 0 streams behind the xtk DMA. Early heads'
              # score+exp for completed kt column chunks are emitted between
              # passes so the Activation engine starts during Kproj.
              n_early = int(os.environ.get("K_EARLY", "16")) if not chunk_pt else 0
              heads_early = tuple(range(n_early)) if (on("v") and on("attn")) else ()
              pt_early = {h: ptpool.tile([128, nk * SQ], BF16, tag="pt",
                                         name=f"pte{h}") for h in heads_early}
              scores_done = set()
              # drip: bound the score/exp burst emitted at each closed-pass
              # boundary so ScalarE digests exps during the next pass's PE
              # work instead of back-pressuring the in-order PE queue.
              score_q = []
              drip_on = os.environ.get("K_DRIP", "1") == "1"
              dripk = int(os.environ.get("K_DRIPK", "12"))
              dripv = int(os.environ.get("K_DRIPV", "5"))

              def drip(n):
                  for _ in range(min(n, len(score_q))):
                      emit_score(*score_q.pop(0))

              def emit_score(h, t):
                  po = 64 * (h % 2)
                  mb = h // 2
                  ss = psS.tile([128, SQ], F32, tag="s", name=f"ss{h}_{t}")
                  nc.tensor.matmul(
                      ss[:],
                      kt_sb[po:po + 64, mb * sk + t * 128: mb * sk + (t + 1) * 128],
                      qt_sb[po:po + 64, mb * SQ:(mb + 1) * SQ],
                      start=True, stop=True)
                  nc.scalar.activation(
                      pt_early[h][:, t * SQ:(t + 1) * SQ], ss[:],
                      mybir.ActivationFunctionType.Exp, scale=0.125)
                  scores_done.add((h, t))

              kt_sb = ktpool.tile([128, nm * sk], BF16, tag="kt")
              nsplits = [(o, min(512, sk - o)) for o in range(0, sk, 512)]
              for (noff, nw) in (nsplits if on("kt") else []):
                  for ms in mgroups:
                      pss = {m: psA.tile([128, 512], F32, tag="proj", name=f"ka{m}") for m in ms}
                      for k in range(nkt):
                          for m in ms:
                              nc.tensor.matmul(
                                  pss[m][:, :nw],
                                  wq_sb[:, k * D + m * 128: k * D + (m + 1) * 128],
                                  xtk_sb[:, k * sk + noff: k * sk + noff + nw],
                                  start=(k == 0), stop=(k == nkt - 1))
                      for m in ms:
                          nc.vector.tensor_copy(
                              kt_sb[:, m * sk + noff: m * sk + noff + nw],
                              pss[m][:, :nw])
                      # score any (early head, tile) whose kt chunk is now done
                      for h in (hh for hh in heads_early if hh // 2 in ms):
                          for t in range(noff // 128,
                                         min(nk, (noff + nw) // 128)):
                              if drip_on:
                                  score_q.append((h, t))
                              else:
                                  emit_score(h, t)
                      if drip_on:
                          drip(dripk)

              # wo loads into the wq slot; Tile serializes on wq's last reader
              wo_sb = wpool.tile([128, nkt * D], BF16, tag="w")
              for k in range(nkt):
                  nc.sync.dma_start(out=wo_sb[:, k * D:(k + 1) * D],
                                    in_=wo_d.ap()[k * 128:(k + 1) * 128, :])

              # ---- V projection into head-interleaved store with mask
              # columns; heads 0/1's score/exp/O work is interleaved into the
              # tile loop so the Activation engine starts ~25us earlier.
              otsplit = oreform and os.environ.get("K_OTSPLIT", "1") == "1"
              if otsplit:
                  # 4-way split so outproj's dependency granularity on ot is
                  # per pair-of-pairs, not the whole store
                  ot_tiles = [otpool.tile([128, 2 * SQ], BF16, tag=f"ot{j}",
                                          name=f"ot{j}")
                              for j in range(nm // 2)]
              else:
                  ot_sb = otpool.tile([128, nm * SQ], BF16, tag="ot")

              def ot_ap(mb, lo, hi):
                  if otsplit:
                      return ot_tiles[mb // 2][:, (mb % 2) * SQ + lo:
                                               (mb % 2) * SQ + hi]
                  return ot_sb[:, mb * SQ + lo: mb * SQ + hi]

              vs_sb = vspool.tile([128, nk * H * VW], BF16, tag="vs")
              for t in range(nk if on("v") else 0):
                  for half in range(2):  # d columns [half*512, half*512+512)
                      ps = psA.tile([128, 512], F32, tag="proj")
                      for k in range(nkt):
                          nc.tensor.matmul(
                              ps[:],
                              xtv_sb[:, k * sk + t * 128: k * sk + (t + 1) * 128],
                              wq_sb[:, k * D + half * 512: k * D + half * 512 + 512],
                              start=(k == 0), stop=(k == nkt - 1))
                      dst = vs_sb[:, t * H * VW + half * 8 * VW:
                                  t * H * VW + (half + 1) * 8 * VW]
                      dst = dst.rearrange("p (h c) -> p h c", c=VW)[:, :, 0:HD]
                      src = ps[:].rearrange("p (h c) -> p h c", c=HD)
                      nc.vector.tensor_scalar_mul(dst, src, vmask_sb[:, t:t + 1])
                      mcols = vs_sb[:, t * H * VW: (t + 1) * H * VW]
                      mcols = mcols.rearrange("p (h c) -> p h c", c=VW)
                      mcols = mcols[:, half * 8:(half + 1) * 8, HD:VW]
                      o16 = ones16[:].rearrange("p (h o) -> p h o", o=1)
                      nc.vector.tensor_scalar_mul(
                          mcols, o16[:, half * 8:(half + 1) * 8, :],
                          vmask_sb[:, t:t + 1])
                      if drip_on:
                          drip(dripv)
                      elif half == 0:
                          for h in heads_early:
                              if (h, t) not in scores_done:
                                  emit_score(h, t)

              if drip_on:
                  drip(len(score_q))

              if oreform:
                  # ---- O + divide + transpose per head pair (2mb, 2mb+1)
                  for mb in range(nm if on("attn") else 0):
                      for h in (2 * mb, 2 * mb + 1):
                          if h not in pt_early:
                              pt_early[h] = ptpool.tile(
                                  [128, nk * SQ], BF16, tag="pt", name=f"ptl{h}")
                              for t in range(nk):
                                  emit_score(h, t)
                      for i in range(SQ // 128):
                          opair = opapool.tile([128, 128], BF16, tag="opair")
                          for h in (2 * mb, 2 * mb + 1):
                              ph = psO.tile([128, VW], F32, tag="o")
                              for t in range(nk):
                                  nc.tensor.matmul(
                                      ph[:],
                                      pt_early[h][:, t * SQ + i * 128:
                                                  t * SQ + (i + 1) * 128],
                                      vs_sb[:, (t * H + h) * VW:
                                            (t * H + h + 1) * VW],
                                      start=(t == 0), stop=(t == nk - 1))
                              inv = invpool.tile([128, 1], F32, tag="inv")
                              if os.environ.get("K_PDIV", "0") == "1":
                                  # stage O to SBUF once (cheap DVE copy),
                                  # then reciprocal reads SBUF and the
                                  # divide multiply runs on the idle Pool
                                  # engine (it cannot access PSUM directly)
                                  oh = invpool.tile([128, VW], F32, tag="oh")
                                  nc.vector.tensor_copy(oh[:], ph[:])
                                  nc.vector.reciprocal(inv[:], oh[:, 64:65])
                                  nc.gpsimd.tensor_scalar_mul(
                                      opair[:, 64 * (h % 2): 64 * (h % 2) + 64],
                                      oh[:, 0:64], inv[:])
                              elif os.environ.get("K_SDIV", "0") == "1":
                                  nc.vector.reciprocal(inv[:], ph[:, 64:65])
                                  nc.scalar.mul(
                                      opair[:, 64 * (h % 2): 64 * (h % 2) + 64],
                                      ph[:, 0:64], inv[:])
                              else:
                                  nc.vector.reciprocal(inv[:], ph[:, 64:65])
                                  nc.vector.tensor_scalar_mul(
                                      opair[:, 64 * (h % 2): 64 * (h % 2) + 64],
                                      ph[:, 0:64], inv[:])
                          pst = psO.tile([128, 128], BF16, tag="o")
                          nc.tensor.transpose(pst[:], opair[:], ident[:])
                          nc.vector.tensor_copy(
                              ot_ap(mb, i * 128, (i + 1) * 128), pst[:])

              for h in heads_early if not oreform else ():
                  po = 64 * (h % 2)
                  mb = h // 2
                  po_ps = psO.tile([VW, SQ], F32, tag="o")
                  for t in range(nk):
                      nc.tensor.matmul(
                          po_ps[:],
                          vs_sb[:, t * H * VW + h * VW: t * H * VW + (h + 1) * VW],
                          pt_early[h][:, t * SQ:(t + 1) * SQ],
                          start=(t == 0), stop=(t == nk - 1))
                  inv = invpool.tile([1, SQ], F32, tag="inv")
                  nc.vector.reciprocal(inv[:], po_ps[64:65, :])
                  inv_rep = invpool.tile([64, SQ], F32, tag="invrep")
                  nc.gpsimd.partition_broadcast(inv_rep[:], inv[:])
                  nc.vector.tensor_mul(
                      ot_sb[po:po + 64, mb * SQ:(mb + 1) * SQ],
                      po_ps[0:64, :], inv_rep[:])

              # ---- attention for the remaining heads
              for h in range(len(heads_early) if (on("attn") and not oreform) else H,
                             H if (on("attn") and not oreform) else 0):
                  po = 64 * (h % 2)       # partition offset of this head's douts
                  mb = h // 2             # dout row-tile holding this head
                  if not chunk_pt:
                      # P^T per head resident; score/exp pass then O^T pass
                      pt = ptpool.tile([128, nk * SQ], BF16, tag="pt")
                      for t in range(nk):
                          ss = psS.tile([128, SQ], F32, tag="s")
                          nc.tensor.matmul(
                              ss[:],
                              kt_sb[po:po + 64, mb * sk + t * 128: mb * sk + (t + 1) * 128],
                              qt_sb[po:po + 64, mb * SQ:(mb + 1) * SQ],
                              start=True, stop=True)
                          nc.scalar.activation(pt[:, t * SQ:(t + 1) * SQ], ss[:],
                                               mybir.ActivationFunctionType.Exp,
                                               scale=0.125)
                      po_ps = psO.tile([VW, SQ], F32, tag="o")
                      for t in range(nk):
                          nc.tensor.matmul(
                              po_ps[:],
                              vs_sb[:, t * H * VW + h * VW: t * H * VW + (h + 1) * VW],
                              pt[:, t * SQ:(t + 1) * SQ],
                              start=(t == 0), stop=(t == nk - 1))
                  else:
                      # chunked P^T (smaller SBUF footprint for large nk)
                      po_ps = psO.tile([VW, SQ], F32, tag="o")
                      for t in range(nk):
                          ss = psS.tile([128, SQ], F32, tag="s")
                          nc.tensor.matmul(
                              ss[:],
                              kt_sb[po:po + 64, mb * sk + t * 128: mb * sk + (t + 1) * 128],
                              qt_sb[po:po + 64, mb * SQ:(mb + 1) * SQ],
                              start=True, stop=True)
                          ptc = ptpool.tile([128, SQ], BF16, tag="pt")
                          nc.scalar.activation(ptc[:], ss[:],
                                               mybir.ActivationFunctionType.Exp,
                                               scale=0.125)
                          nc.tensor.matmul(
                              po_ps[:],
                              vs_sb[:, t * H * VW + h * VW: t * H * VW + (h + 1) * VW],
                              ptc[:],
                              start=(t == 0), stop=(t == nk - 1),
                              skip_group_check=True)
                  inv = invpool.tile([1, SQ], F32, tag="inv")
                  nc.vector.reciprocal(inv[:], po_ps[64:65, :])
                  # broadcast inv to 64 partitions on the (idle) gpsimd engine
                  inv_rep = invpool.tile([64, SQ], F32, tag="invrep")
                  nc.gpsimd.partition_broadcast(inv_rep[:], inv[:])
                  nc.vector.tensor_mul(
                      ot_sb[po:po + 64, mb * SQ:(mb + 1) * SQ],
                      po_ps[0:64, :], inv_rep[:])

              # ---- output projection: outT[dout, sq] = Wo^T-tiles @ O^T
              # out DMAs batched 4 m-tiles at a time to amortize issue latency
              osb4 = None
              for m in range(nm if on("out") else 0):
                  ps = psA.tile([128, SQ], F32, tag="proj")
                  for k in range(nkt):
                      nc.tensor.matmul(
                          ps[:],
                          wo_sb[:, k * D + m * 128: k * D + (m + 1) * 128],
                          ot_ap(k, 0, SQ),
                          start=(k == 0), stop=(k == nkt - 1))
                  if m in (0, 4, 7):
                      bw = {0: 4, 4: 3, 7: 1}[m]
                      mbase = m
                      osb4 = osbpool.tile([128, 4 * SQ],
                                          BF16 if obf else F32, tag="outsb")
                  if os.environ.get("K_SOUT", "0") == "1":
                      nc.scalar.copy(
                          osb4[:, (m - mbase) * SQ:(m - mbase + 1) * SQ], ps[:])
                  else:
                      nc.vector.tensor_copy(
                          osb4[:, (m - mbase) * SQ:(m - mbase + 1) * SQ], ps[:])
                  if m == mbase + bw - 1:
                      dst = out_d.ap()[mbase * 128:(m + 1) * 128, :]
                      dst = dst.rearrange("(mm p) c -> p mm c", p=128)
                      nc.sync.dma_start(out=dst, in_=osb4[:, :bw * SQ])

    nc.compile()
    return nc


def exch_enabled() -> bool:
    return os.environ.get("K_EXCH", "0") == "1"


def make_in_maps(queries, keys, values, vls, W_q, W_o, nk):
    """Per-core input tensors matching the active module variant."""
    sk = nk * 128
    wq16 = np.ascontiguousarray(np.asarray(W_q, np.float32).astype(BF16_NP))
    wo16 = np.ascontiguousarray(np.asarray(W_o, np.float32).astype(BF16_NP))
    exch = exch_enabled()
    nkp = nk + (nk % 2)
    hk = nkp // 2
    sk2 = hk * 128
    in_maps = []
    for c in range(N_CORES):
        b, r = c // 2, c % 2
        vl = int(vls[b])
        if exch:
            lo = r * sk2
            hi = min(sk, lo + sk2)
            xk = np.zeros((D, sk2), dtype=BF16_NP)
            xv = np.zeros((D, sk2), dtype=BF16_NP)
            if hi > lo:
                xk[:, 0:hi - lo] = keys[b, lo:hi, :].T.astype(BF16_NP)
                xv[:, 0:hi - lo] = values[b, lo:hi, :].T.astype(BF16_NP)
            vm = (np.arange(lo, lo + sk2) < vl).astype(np.float32)
            vm = vm.reshape(hk, 128).T
        else:
            xk = keys[b, :sk, :].T.astype(BF16_NP)
            xv = values[b, :sk, :].T.astype(BF16_NP)
            vm = (np.arange(sk) < vl).astype(np.float32).reshape(nk, 128).T
        in_maps.append({
            "xtq": np.ascontiguousarray(
                queries[b, r * SQ:(r + 1) * SQ, :].T.astype(BF16_NP)),
            "xtk": np.ascontiguousarray(xk),
            "xtv": np.ascontiguousarray(xv),
            "wq": wq16,
            "wo": wo16,
            "vmask": np.ascontiguousarray(vm),
        })
    return in_maps


def get_module(nk: int):
    key = (nk, exch_enabled())
    nc = _module_cache.get(key)
    if nc is None:
        nc = _build_module_exch(nk) if exch_enabled() else _build_module(nk)
        _module_cache[key] = nc
    return nc


def build_for_bench(nk: int, reps: int = 1):
    """Fresh (uncached) module for the active variant; used by test harness."""
    if exch_enabled():
        return _build_module_exch(nk, reps=reps)
    return _build_module(nk, reps=reps)


def kernel(queries, keys, values, valid_lengths, W_q, W_o):
    queries = np.ascontiguousarray(np.asarray(queries, dtype=np.float32))
    keys = np.ascontiguousarray(np.asarray(keys, dtype=np.float32))
    values = np.ascontiguousarray(np.asarray(values, dtype=np.float32))
    W_q = np.ascontiguousarray(np.asarray(W_q, dtype=np.float32))
    W_o = np.ascontiguousarray(np.asarray(W_o, dtype=np.float32))
    vls = np.asarray(valid_lengths).astype(np.int64)

    nk = max(1, int(-(-int(vls.max()) // 128)))  # ceil(max_vl/128)

    nc = get_module(nk)
    in_maps = make_in_maps(queries, keys, values, vls, W_q, W_o, nk)

    res = run_bass_kernel_spmd(nc, in_maps, list(range(N_CORES)))

    out = np.empty((B, S, D), dtype=np.float32)
    for c in range(N_CORES):
        b, r = c // 2, c % 2
        out[b, r * SQ:(r + 1) * SQ, :] = \
            res.results[c]["outT"].T.astype(np.float32)
    return out

